# revision 20
# baseline (speedup 1.0000x reference)
"""CenterLoss kernel for Trainium2 (8 NeuronCores, data-parallel over batch).

reference: mean(clip(distmat[i, labels[i]])) where
  distmat[i,c] = ||x_i||^2 + ||c_c||^2 - 2 x_i . c_c
i.e. the loss only needs dist_i = ||x_i - centers[labels[i]]||^2 — a gather +
elementwise + reduce; the full (N, C) matmul in the reference is dead work.
The clip is provably inactive for this problem (distances are O(1e3), far from
1e-12/1e12), so the mean only needs per-partition sums, not per-row values.

Per core (512 rows of the 4096-row batch), same expansion as the reference:
  - one dma_gather brings all 512 centers rows in a single SWDGE instruction
    (out[p, j] = centers[idx[j*128+p]]); indices ride as an int16 [128, 32]
    tile (16-partition wrap).  One instruction means one completion sem —
    SWDGE sems lag their data by ~1-3us, so fewer is faster than pipelining.
  - x enters SBUF slot-major ([p, j] = batch row j*128+p) via the two HWDGE
    queues (sync + scalar).
  - ScalarE: sum(x^2) halves early + three sum(c^2) columns; VectorE: the
    other sum(x^2) half early + four sum(x*c) columns + the last sum(c^2)
    (as x*x-style scalar_tensor_tensor, which needs no accumulator-read).
  - A [128, 12] accumulator tile DMAs out; the host forms
    sum(xsq) + sum(csq) - 2*sum(xc) over everything and divides by N.
"""

import os

import numpy as np

# clears a wedged NeuronCore from a previous crashed run at NRT init
os.environ.setdefault("NEURON_RT_RESET_CORES", "1")

N, D, C = 4096, 512, 10000
NCORES = 8
ROWS_PER_CORE = N // NCORES  # 512
P = 128
J = ROWS_PER_CORE // P  # 4 rows per partition
IDX_COLS = ROWS_PER_CORE // 16  # int16 index tile free dim (16-partition wrap)

CLAMP = 1e-12

_cache = {}

SCRATCH_SIZE = 65536  # SWDGE descriptor ring


def _build_nc():
    import concourse.mybir as mybir
    from concourse import bacc
    from concourse.tile import TileContext

    nc = bacc.Bacc(
        "TRN2",
        target_bir_lowering=False,
        debug=False,
        num_devices=NCORES,
        # 512 gather descriptor pairs x 64B stream through the SWDGE ring
        dynamic_dma_scratch_size=SCRATCH_SIZE,
    )
    x = nc.dram_tensor("x", [P, J * D], mybir.dt.float32, kind="ExternalInput")
    labels = nc.dram_tensor("labels", [P, IDX_COLS], mybir.dt.int16, kind="ExternalInput")
    centers = nc.dram_tensor("centers", [C, D], mybir.dt.float32, kind="ExternalInput")
    # columns: [0, J) = sum(x^2), [J, 2J) = sum(c^2), [2J, 3J) = sum(x*c)
    out = nc.dram_tensor("out", [P, 3 * J], mybir.dt.float32, kind="ExternalOutput")

    with TileContext(nc) as tc:
        with (
            tc.tile_pool(name="io", bufs=1) as io_pool,
            tc.tile_pool(name="work", bufs=1) as work,
        ):
            # labels first on sync — the gather is gated on it
            lab_tile = io_pool.tile([P, IDX_COLS], mybir.dt.int16, tag="lab")
            nc.sync.dma_start(out=lab_tile[:], in_=labels[:])

            # per-chunk x tiles; loads split across the two HWDGE queues
            # (scalar gets chunk 0 so it isn't queued behind the labels DMA)
            xts = []
            hw_engs = [nc.scalar, nc.sync]
            for j in range(J):
                xt = io_pool.tile([P, D], mybir.dt.float32, tag=f"x{j}")
                xts.append(xt)
                hw_engs[j % 2].dma_start(out=xt[:], in_=x[:, j * D : (j + 1) * D])

            acc = io_pool.tile([P, 3 * J], mybir.dt.float32, tag="acc")

            # sum(x^2) — runs while the gather streams in
            for j, xt in enumerate(xts):
                if j % 2 == 0:
                    sq = work.tile([P, D], mybir.dt.float32, tag=f"wsa{j}")
                    nc.scalar.activation(
                        out=sq[:],
                        in_=xt[:],
                        func=mybir.ActivationFunctionType.Square,
                        accum_out=acc[:, j : j + 1],
                    )
                else:
                    sq = work.tile([P, D], mybir.dt.float32, tag=f"wsv{j}")
                    nc.vector.scalar_tensor_tensor(
                        out=sq[:],
                        in0=xt[:],
                        scalar=0.0,
                        in1=xt[:],
                        op0=mybir.AluOpType.add,
                        op1=mybir.AluOpType.mult,
                        accum_out=acc[:, j : j + 1],
                    )

            # all 512 rows in one SWDGE gather: gt[p, j, :] = centers[idx[j*128+p]]
            gt = io_pool.tile([P, J, D], mybir.dt.float32, tag="g")
            nc.gpsimd.dma_gather(
                gt[:],
                centers[:],
                lab_tile[:],
                ROWS_PER_CORE,
                ROWS_PER_CORE,
                D,
            )

            # gather-gated reduces, balanced: ScalarE csq0-2 (+accum reads),
            # VectorE xc0-3 + csq3 (STT needs no read)
            for j, xt in enumerate(xts):
                gsl = gt[:, j, :]
                if j < 3:
                    sq = work.tile([P, D], mybir.dt.float32, tag=f"wca{j}")
                    nc.scalar.activation(
                        out=sq[:],
                        in_=gsl,
                        func=mybir.ActivationFunctionType.Square,
                        accum_out=acc[:, J + j : J + j + 1],
                    )
                xc = work.tile([P, D], mybir.dt.float32, tag=f"wxc{j}")
                nc.vector.scalar_tensor_tensor(
                    out=xc[:],
                    in0=xt[:],
                    scalar=0.0,
                    in1=gsl,
                    op0=mybir.AluOpType.add,
                    op1=mybir.AluOpType.mult,
                    accum_out=acc[:, 2 * J + j : 2 * J + j + 1],
                )
            csq3 = work.tile([P, D], mybir.dt.float32, tag="wca3")
            nc.vector.scalar_tensor_tensor(
                out=csq3[:],
                in0=gt[:, 3, :],
                scalar=0.0,
                in1=gt[:, 3, :],
                op0=mybir.AluOpType.add,
                op1=mybir.AluOpType.mult,
                accum_out=acc[:, J + 3 : J + 4],
            )

            nc.sync.dma_start(out=out[:], in_=acc[:])

    nc.compile()
    return nc


def _run(in_maps, trace=False):
    from concourse.bass_utils import run_bass_kernel_spmd

    if "nc" not in _cache:
        _cache["nc"] = _build_nc()
    return run_bass_kernel_spmd(
        _cache["nc"], in_maps, list(range(NCORES)), trace=trace
    )


def kernel(x, labels, centers, _trace=False):
    x = np.ascontiguousarray(np.asarray(x, dtype=np.float32))
    labels = np.asarray(labels).astype(np.int16)
    centers = np.ascontiguousarray(np.asarray(centers, dtype=np.float32))

    R = ROWS_PER_CORE
    in_maps = []
    for c in range(NCORES):
        lo = c * R
        hi = lo + R
        # slot-major layout: slot s = j*128 + p holds batch row lo + s, so
        # x_tile[p, j*D:(j+1)*D] = x[lo + j*128 + p]
        xs = (
            x[lo:hi]
            .reshape(J, P, D)
            .transpose(1, 0, 2)
            .reshape(P, J * D)
        )
        # dma_gather reads flat index i from wrapped[i % 16, i // 16]
        lw = labels[lo:hi].reshape(IDX_COLS, 16).T  # [16, IDX_COLS]
        lab16 = np.ascontiguousarray(np.tile(lw, (P // 16, 1)))  # [128, IDX_COLS]
        in_maps.append(
            {
                "x": np.ascontiguousarray(xs),
                "labels": lab16,
                "centers": centers,
            }
        )

    res = _run(in_maps, trace=_trace)
    total = 0.0
    for c in range(NCORES):
        a = np.asarray(res.results[c]["out"], dtype=np.float64)  # [P, 3J]
        total += a[:, : 2 * J].sum() - 2.0 * a[:, 2 * J :].sum()
    # the clip is inactive for these inputs (dist >> 1e-12), so mean(clip(d))
    # == sum(d)/N
    loss = total / N
    out = np.asarray(loss, dtype=np.float32)
    if _trace:
        return out, res
    return out
